# revision 14
# baseline (speedup 1.0000x reference)
"""CBAM-style spatial attention kernel for Trainium2 (Bass/Tile), SPMD over 8 cores.

Computation (per batch b):
    avg = mean(y[b], axis=C)            # [H, W]
    mx  = max(y[b], axis=C)             # [H, W]
    gate = sigmoid(conv2d([avg; mx], conv_w, pad=1))   # [H, W]
    out[b] = x[b] * gate                # broadcast over C

Sharding: data-parallel on batch, 4 batches per core, conv_w replicated.

Per-core kernel design:
  - y streamed as [128ch, 2048pix] tiles (2 channel chunks). Each 128-pixel
    subtile is pushed through a PE matmul with moving operand [I | ones/256]
    (129 cols): cols 0..127 give the transpose ([pix, ch] in PSUM), col 128
    gives the per-pixel partial channel mean for free.
  - DVE does one XY max-reduce over the two transposed chunks and one tiny
    X-reduce over the two partial means; planes land transposed (plane_T[w,h]).
  - 3x3 conv = 6 accumulating PE matmuls with banded stationary matrices
    built at runtime from conv_w (dx via band diagonals, dy via shifted
    column windows of the zero-padded planes). Sigmoid on ACT.
  - Gate is PE-transposed back to [h, w], gathered to sig8[8, 2048] (one
    f-tile per partition), then DMA partition-broadcast (step-0) into
    [128, 2048] tiles for the elementwise multiply (DVE for channel chunk A,
    GpSimd for chunk B).
  - DMA queue split to avoid head-of-line blocking: y loads on SP HWDGE,
    x loads on ACT HWDGE, stores + gate broadcasts on Pool SWDGE.
"""

import numpy as np

_B = 32
_NCORES = 8
_BPC = _B // _NCORES  # 4
_C = 256
_H = 128
_W = 128
_HW = _H * _W
_FT = 2048  # pixels per streaming tile
_NFT = _HW // _FT  # 8
_SUBS = _FT // 128  # 16

_NC_CACHE = {}


def _build(bpc):
    from contextlib import ExitStack

    import concourse.bass as bass
    import concourse.tile as tile
    from concourse import bacc, mybir

    f32 = mybir.dt.float32
    Alu = mybir.AluOpType
    Act = mybir.ActivationFunctionType

    nc = bacc.Bacc("TRN2", target_bir_lowering=False)
    xh = nc.dram_tensor("x", (bpc, _C, _H, _W), f32, kind="ExternalInput")
    yh = nc.dram_tensor("y", (bpc, _C, _H, _W), f32, kind="ExternalInput")
    wh = nc.dram_tensor("conv_w", (1, 2, 3, 3), f32, kind="ExternalInput")
    oh = nc.dram_tensor("out", (bpc, _C, _H, _W), f32, kind="ExternalOutput")
    sig_dram = nc.dram_tensor("sig_scratch", (bpc, _HW), f32)
    sig_ap = sig_dram[:]

    xr = xh[:].rearrange("b c h w -> b c (h w)")
    yr = yh[:].rearrange("b c h w -> b c (h w)")
    orr = oh[:].rearrange("b c h w -> b c (h w)")

    def pbcast(ap, n=128):
        """Partition-broadcast (step-0) view of a [1, F] AP."""
        return bass.AP(
            tensor=ap.tensor,
            offset=ap.offset,
            ap=[[0, n]] + [list(p) for p in ap.ap[1:]],
        )

    with tile.TileContext(nc) as tc, ExitStack() as ctx:
        consts = ctx.enter_context(tc.tile_pool(name="consts", bufs=1))
        ypool = ctx.enter_context(tc.tile_pool(name="ypool", bufs=7))
        xpool = ctx.enter_context(tc.tile_pool(name="xpool", bufs=7))
        gbpool = ctx.enter_context(tc.tile_pool(name="gbpool", bufs=3))
        planep = ctx.enter_context(tc.tile_pool(name="planep", bufs=2))
        smallp = ctx.enter_context(tc.tile_pool(name="smallp", bufs=2))
        scrp = ctx.enter_context(tc.tile_pool(name="scrp", bufs=2))
        tpsum = ctx.enter_context(
            tc.tile_pool(name="tpsum", bufs=4, space=bass.MemorySpace.PSUM)
        )
        gpsum = ctx.enter_context(
            tc.tile_pool(name="gpsum", bufs=2, space=bass.MemorySpace.PSUM)
        )

        # ---- constants ----
        from concourse.masks import make_identity

        identity_t = consts.tile([128, 128], f32)
        make_identity(nc, identity_t[:])
        identity = identity_t[:]

        # conv_w broadcast to all partitions: wbc[p, i] = w.flat[i]
        wbc = consts.tile([128, 18], f32)
        w_flat = wh[:].rearrange("o i kh kw -> o (i kh kw)")  # [1, 18]
        nc.gpsimd.dma_start(out=wbc[:], in_=pbcast(w_flat))

        # diagonal shift masks: masks_t[:, j, :][p, f] = 1.0 iff f - p == j - 1
        masks_t = consts.tile([128, 3, 128], f32)
        for j_i, j in enumerate((-1, 0, 1)):
            m = masks_t[:, j_i, :]
            nc.gpsimd.memset(m, 0.0)
            nc.gpsimd.affine_select(
                out=m,
                in_=m,
                compare_op=Alu.not_equal,
                fill=1.0,
                base=-j,
                channel_multiplier=-1,
                pattern=[[1, 128]],
            )

        # banded stationary matrices: band[:, c*3+ky, :][w', w] = W[c, ky, (w-w')+1]
        band = consts.tile([128, 6, 128], f32)
        scr_band = consts.tile([128, 128], f32)
        for c in range(2):
            for ky in range(3):
                idx = c * 3 + ky
                base_i = c * 9 + ky * 3
                # band[p=w', f=w] = W[c, ky, kx] with kx = (p - f) + 1,
                # i.e. mask j = f - p pairs with kx = 1 - j
                bsl = band[:, idx, :]
                nc.vector.tensor_scalar_mul(
                    bsl, masks_t[:, 0, :], wbc[:, base_i + 2 : base_i + 3]
                )
                for j_i, kx in ((1, 1), (2, 0)):
                    nc.vector.tensor_scalar_mul(
                        scr_band[:],
                        masks_t[:, j_i, :],
                        wbc[:, base_i + kx : base_i + kx + 1],
                    )
                    nc.vector.tensor_add(bsl, bsl, scr_band[:])

        # ---- per-batch pipeline ----
        for b in range(bpc):
            # channel-reduced planes, transposed: plane[w, h+1]; cols 0 and H+1 zero
            avgT = planep.tile([128, _H + 2], f32, tag="avgT")
            maxT = planep.tile([128, _H + 2], f32, tag="maxT")
            for t in (avgT, maxT):
                nc.gpsimd.memset(t[:, 0:1], 0.0)
                nc.gpsimd.memset(t[:, _H + 1 : _H + 2], 0.0)

            for ft in range(_NFT):
                sl = slice(ft * _FT, (ft + 1) * _FT)
                ya = ypool.tile([128, _FT], f32, tag="y")
                yb = ypool.tile([128, _FT], f32, tag="y")
                nc.sync.dma_start(out=ya[:], in_=yr[b, 0:128, sl])
                nc.sync.dma_start(out=yb[:], in_=yr[b, 128:256, sl])
                for s in range(_SUBS):
                    hcol = ft * _SUBS + s + 1  # plane column = h + 1
                    pt = tpsum.tile([128, 256], f32, tag="pt")
                    nc.tensor.transpose(
                        pt[:, 0:128], ya[:, s * 128 : (s + 1) * 128], identity
                    )
                    nc.tensor.transpose(
                        pt[:, 128:256], yb[:, s * 128 : (s + 1) * 128], identity
                    )
                    nc.vector.tensor_reduce(
                        out=maxT[:, hcol : hcol + 1],
                        in_=pt[:],
                        axis=mybir.AxisListType.X,
                        op=Alu.max,
                    )
                    ascr = scrp.tile([128, 256], f32, tag="ascr")
                    nc.scalar.activation(
                        out=ascr[:],
                        in_=pt[:],
                        func=Act.Copy,
                        scale=1.0 / _C,
                        accum_out=avgT[:, hcol : hcol + 1],
                    )

            # 3x3 conv over [avg; max] planes -> gate_T[w, h] in PSUM
            gate_ps = gpsum.tile([128, 128], f32, tag="gate")
            planes = (avgT, maxT)
            for c in range(2):
                for ky in range(3):
                    idx = c * 3 + ky
                    nc.tensor.matmul(
                        gate_ps[:],
                        band[:, idx, :],
                        planes[c][:, ky : ky + _H],
                        start=(idx == 0),
                        stop=(idx == 5),
                    )
            sigT = smallp.tile([128, 128], f32, tag="sigT")
            nc.scalar.activation(out=sigT[:], in_=gate_ps[:], func=Act.Sigmoid)
            sig_ps = gpsum.tile([128, 128], f32, tag="sigps")
            nc.tensor.matmul(sig_ps[:], sigT[:], identity, start=True, stop=True)
            sighw = smallp.tile([128, 128], f32, tag="sighw")
            nc.scalar.copy(out=sighw[:], in_=sig_ps[:])
            nc.gpsimd.dma_start(out=sig_ap[b, :], in_=sighw[:])

            # multiply: out[b] = x[b] * gate (gate DMA-broadcast across partitions)
            for ft in range(_NFT):
                sl = slice(ft * _FT, (ft + 1) * _FT)
                gb = gbpool.tile([128, _FT], f32, tag="gb")
                nc.gpsimd.dma_start(out=gb[:], in_=pbcast(sig_ap[b : b + 1, sl]))
                xa = xpool.tile([128, _FT], f32, tag="x")
                xb2 = xpool.tile([128, _FT], f32, tag="x")
                nc.scalar.dma_start(out=xa[:], in_=xr[b, 0:128, sl])
                nc.scalar.dma_start(out=xb2[:], in_=xr[b, 128:256, sl])
                nc.vector.tensor_mul(xa[:], xa[:], gb[:])
                nc.vector.tensor_mul(xb2[:], xb2[:], gb[:])
                nc.gpsimd.dma_start(out=orr[b, 0:128, sl], in_=xa[:])
                nc.gpsimd.dma_start(out=orr[b, 128:256, sl], in_=xb2[:])

    nc.compile()
    return nc


def _get_nc(bpc=_BPC):
    if bpc not in _NC_CACHE:
        _NC_CACHE[bpc] = _build(bpc)
    return _NC_CACHE[bpc]


def kernel(x, y, conv_w):
    x = np.ascontiguousarray(np.asarray(x, dtype=np.float32))
    y = np.ascontiguousarray(np.asarray(y, dtype=np.float32))
    conv_w = np.ascontiguousarray(np.asarray(conv_w, dtype=np.float32))
    assert x.shape == (_B, _C, _H, _W), x.shape
    assert y.shape == x.shape
    assert conv_w.shape == (1, 2, 3, 3)

    from concourse.bass_utils import run_bass_kernel_spmd

    nc = _get_nc()
    in_maps = [
        {
            "x": x[i * _BPC : (i + 1) * _BPC],
            "y": y[i * _BPC : (i + 1) * _BPC],
            "conv_w": conv_w,
        }
        for i in range(_NCORES)
    ]
    res = run_bass_kernel_spmd(nc, in_maps, list(range(_NCORES)))
    return np.concatenate([r["out"] for r in res.results], axis=0)


if __name__ == "__main__":
    # quick single-core CoreSim check with bpc=1
    from concourse.bass_interp import CoreSim

    bpc = 1
    nc = _get_nc(bpc)
    rng = np.random.default_rng(0)
    x = rng.standard_normal((bpc, _C, _H, _W), dtype=np.float32)
    y = rng.standard_normal((bpc, _C, _H, _W), dtype=np.float32)
    w = rng.uniform(-0.2, 0.2, (1, 2, 3, 3)).astype(np.float32)

    sim = CoreSim(nc)
    sim.tensor("x")[:] = x
    sim.tensor("y")[:] = y
    sim.tensor("conv_w")[:] = w
    sim.simulate()
    got = sim.tensor("out").copy()

    # numpy reference
    avg = y.mean(axis=1, keepdims=True)
    mx = y.max(axis=1, keepdims=True)
    s = np.concatenate([avg, mx], axis=1)  # [b, 2, H, W]
    sp = np.pad(s, ((0, 0), (0, 0), (1, 1), (1, 1)))
    gate = np.zeros((bpc, _H, _W), dtype=np.float32)
    for ky in range(3):
        for kx in range(3):
            for c in range(2):
                gate += w[0, c, ky, kx] * sp[:, c, ky : ky + _H, kx : kx + _W]
    gate = 1.0 / (1.0 + np.exp(-gate))
    want = x * gate[:, None, :, :]

    err = np.abs(got - want).max()
    rel = err / np.abs(want).max()
    print("max abs err:", err, "rel:", rel)


# revision 18
# speedup vs baseline: 1.0328x; 1.0328x over previous
"""CBAM-style spatial attention kernel for Trainium2 (Bass/Tile), SPMD over 8 cores.

Computation (per batch b):
    avg = mean(y[b], axis=C)            # [H, W]
    mx  = max(y[b], axis=C)             # [H, W]
    gate = sigmoid(conv2d([avg; mx], conv_w, pad=1))   # [H, W]
    out[b] = x[b] * gate                # broadcast over C

Sharding: data-parallel on batch, 4 batches per core, conv_w replicated.

Per-core kernel design:
  - y streamed as [128ch, 2048pix] tiles (2 channel chunks). Each 128-pixel
    subtile is pushed through a PE matmul with moving operand [I | ones/256]
    (129 cols): cols 0..127 give the transpose ([pix, ch] in PSUM), col 128
    gives the per-pixel partial channel mean for free.
  - DVE does one XY max-reduce over the two transposed chunks and one tiny
    X-reduce over the two partial means; planes land transposed (plane_T[w,h]).
  - 3x3 conv = 6 accumulating PE matmuls with banded stationary matrices
    built at runtime from conv_w (dx via band diagonals, dy via shifted
    column windows of the zero-padded planes). Sigmoid on ACT.
  - Gate is PE-transposed back to [h, w], gathered to sig8[8, 2048] (one
    f-tile per partition), then DMA partition-broadcast (step-0) into
    [128, 2048] tiles for the elementwise multiply (DVE for channel chunk A,
    GpSimd for chunk B).
  - DMA queue split to avoid head-of-line blocking: y loads on SP HWDGE,
    x loads on ACT HWDGE, stores + gate broadcasts on Pool SWDGE.
"""

import numpy as np

_B = 32
_NCORES = 8
_BPC = _B // _NCORES  # 4
_C = 256
_H = 128
_W = 128
_HW = _H * _W
_FT = 2048  # pixels per streaming tile
_NFT = _HW // _FT  # 8
_SUBS = _FT // 128  # 16
_GRP = 4  # subtiles per wide PSUM reduce group

_NC_CACHE = {}


def _build(bpc):
    from contextlib import ExitStack

    import concourse.bass as bass
    import concourse.tile as tile
    from concourse import bacc, mybir

    f32 = mybir.dt.float32
    Alu = mybir.AluOpType
    Act = mybir.ActivationFunctionType

    nc = bacc.Bacc("TRN2", target_bir_lowering=False)
    xh = nc.dram_tensor("x", (bpc, _C, _H, _W), f32, kind="ExternalInput")
    yh = nc.dram_tensor("y", (bpc, _C, _H, _W), f32, kind="ExternalInput")
    wh = nc.dram_tensor("conv_w", (1, 2, 3, 3), f32, kind="ExternalInput")
    oh = nc.dram_tensor("out", (bpc, _C, _H, _W), f32, kind="ExternalOutput")
    sig_dram = nc.dram_tensor("sig_scratch", (bpc, _HW), f32)
    sig_ap = sig_dram[:]

    xr = xh[:].rearrange("b c h w -> b c (h w)")
    yr = yh[:].rearrange("b c h w -> b c (h w)")
    orr = oh[:].rearrange("b c h w -> b c (h w)")

    def pbcast(ap, n=128):
        """Partition-broadcast (step-0) view of a [1, F] AP."""
        return bass.AP(
            tensor=ap.tensor,
            offset=ap.offset,
            ap=[[0, n]] + [list(p) for p in ap.ap[1:]],
        )

    with tile.TileContext(nc) as tc, ExitStack() as ctx:
        consts = ctx.enter_context(tc.tile_pool(name="consts", bufs=1))
        ypool = ctx.enter_context(tc.tile_pool(name="ypool", bufs=7))
        xpool = ctx.enter_context(tc.tile_pool(name="xpool", bufs=7))
        gbpool = ctx.enter_context(tc.tile_pool(name="gbpool", bufs=3))
        planep = ctx.enter_context(tc.tile_pool(name="planep", bufs=2))
        smallp = ctx.enter_context(tc.tile_pool(name="smallp", bufs=2))
        tpsum = ctx.enter_context(
            tc.tile_pool(name="tpsum", bufs=3, space=bass.MemorySpace.PSUM)
        )
        gpsum = ctx.enter_context(
            tc.tile_pool(name="gpsum", bufs=1, space=bass.MemorySpace.PSUM)
        )

        # ---- constants ----
        from concourse.masks import make_identity

        identity_t = consts.tile([128, 128], f32)
        make_identity(nc, identity_t[:])
        identity = identity_t[:]

        # conv_w broadcast to all partitions: wbc[p, i] = w.flat[i]
        wbc = consts.tile([128, 18], f32)
        w_flat = wh[:].rearrange("o i kh kw -> o (i kh kw)")  # [1, 18]
        nc.gpsimd.dma_start(out=wbc[:], in_=pbcast(w_flat))
        # avg plane holds the channel SUM; fold mean's 1/C into the avg weights
        nc.scalar.mul(wbc[:, 0:9], wbc[:, 0:9], 1.0 / _C)

        # diagonal shift masks: masks_t[:, j, :][p, f] = 1.0 iff f - p == j - 1
        masks_t = consts.tile([128, 3, 128], f32)
        for j_i, j in enumerate((-1, 0, 1)):
            m = masks_t[:, j_i, :]
            nc.gpsimd.memset(m, 0.0)
            nc.gpsimd.affine_select(
                out=m,
                in_=m,
                compare_op=Alu.not_equal,
                fill=1.0,
                base=-j,
                channel_multiplier=-1,
                pattern=[[1, 128]],
            )

        # banded stationary matrices: band[:, c*3+ky, :][w', w] = W[c, ky, (w-w')+1]
        band = consts.tile([128, 6, 128], f32)
        scr_band = consts.tile([128, 128], f32)
        for c in range(2):
            for ky in range(3):
                idx = c * 3 + ky
                base_i = c * 9 + ky * 3
                # band[p=w', f=w] = W[c, ky, kx] with kx = (p - f) + 1,
                # i.e. mask j = f - p pairs with kx = 1 - j
                bsl = band[:, idx, :]
                nc.vector.tensor_scalar_mul(
                    bsl, masks_t[:, 0, :], wbc[:, base_i + 2 : base_i + 3]
                )
                for j_i, kx in ((1, 1), (2, 0)):
                    nc.vector.tensor_scalar_mul(
                        scr_band[:],
                        masks_t[:, j_i, :],
                        wbc[:, base_i + kx : base_i + kx + 1],
                    )
                    nc.vector.tensor_add(bsl, bsl, scr_band[:])

        # ---- per-batch pipeline ----
        for b in range(bpc):
            # channel-reduced planes, transposed: plane[w, h+1]; cols 0 and H+1 zero
            avgT = planep.tile([128, _H + 2], f32, tag="avgT")
            maxT = planep.tile([128, _H + 2], f32, tag="maxT")
            for t in (avgT, maxT):
                nc.gpsimd.memset(t[:, 0:1], 0.0)
                nc.gpsimd.memset(t[:, _H + 1 : _H + 2], 0.0)

            for ft in range(_NFT):
                sl = slice(ft * _FT, (ft + 1) * _FT)
                ya = ypool.tile([128, _FT], f32, tag="y")
                yb = ypool.tile([128, _FT], f32, tag="y")
                nc.sync.dma_start(out=ya[:], in_=yr[b, 0:128, sl])
                nc.sync.dma_start(out=yb[:], in_=yr[b, 128:256, sl])
                for g in range(_SUBS // _GRP):
                    hcol = ft * _SUBS + g * _GRP + 1  # plane column = h + 1
                    pt = tpsum.tile([128, _GRP, 256], f32, tag="pt")
                    for s_i in range(_GRP):
                        s = g * _GRP + s_i
                        nc.tensor.transpose(
                            pt[:, s_i, 0:128],
                            ya[:, s * 128 : (s + 1) * 128],
                            identity,
                        )
                        nc.tensor.transpose(
                            pt[:, s_i, 128:256],
                            yb[:, s * 128 : (s + 1) * 128],
                            identity,
                        )
                    nc.vector.tensor_reduce(
                        out=maxT[:, hcol : hcol + _GRP],
                        in_=pt[:],
                        axis=mybir.AxisListType.X,
                        op=Alu.max,
                    )
                    nc.vector.tensor_reduce(
                        out=avgT[:, hcol : hcol + _GRP],
                        in_=pt[:],
                        axis=mybir.AxisListType.X,
                        op=Alu.add,
                    )

            # 3x3 conv over [avg; max] planes -> gate_T[w, h] in PSUM
            gate_ps = gpsum.tile([128, 128], f32, tag="gate")
            planes = (avgT, maxT)
            for c in range(2):
                for ky in range(3):
                    idx = c * 3 + ky
                    nc.tensor.matmul(
                        gate_ps[:],
                        band[:, idx, :],
                        planes[c][:, ky : ky + _H],
                        start=(idx == 0),
                        stop=(idx == 5),
                    )
            sigT = smallp.tile([128, 128], f32, tag="sigT")
            nc.scalar.activation(out=sigT[:], in_=gate_ps[:], func=Act.Sigmoid)
            sig_ps = gpsum.tile([128, 128], f32, tag="sigps")
            nc.tensor.matmul(sig_ps[:], sigT[:], identity, start=True, stop=True)
            sighw = smallp.tile([128, 128], f32, tag="sighw")
            nc.scalar.copy(out=sighw[:], in_=sig_ps[:])
            nc.gpsimd.dma_start(out=sig_ap[b, :], in_=sighw[:])

            # multiply: out[b] = x[b] * gate (gate DMA-broadcast across partitions)
            for ft in range(_NFT):
                sl = slice(ft * _FT, (ft + 1) * _FT)
                gb = gbpool.tile([128, _FT], f32, tag="gb")
                nc.gpsimd.dma_start(out=gb[:], in_=pbcast(sig_ap[b : b + 1, sl]))
                xa = xpool.tile([128, _FT], f32, tag="x")
                xb2 = xpool.tile([128, _FT], f32, tag="x")
                nc.scalar.dma_start(out=xa[:], in_=xr[b, 0:128, sl])
                nc.scalar.dma_start(out=xb2[:], in_=xr[b, 128:256, sl])
                nc.vector.tensor_mul(xa[:], xa[:], gb[:])
                nc.vector.tensor_mul(xb2[:], xb2[:], gb[:])
                nc.gpsimd.dma_start(out=orr[b, 0:128, sl], in_=xa[:])
                nc.gpsimd.dma_start(out=orr[b, 128:256, sl], in_=xb2[:])

    nc.compile()
    return nc


def _get_nc(bpc=_BPC):
    if bpc not in _NC_CACHE:
        _NC_CACHE[bpc] = _build(bpc)
    return _NC_CACHE[bpc]


def kernel(x, y, conv_w):
    x = np.ascontiguousarray(np.asarray(x, dtype=np.float32))
    y = np.ascontiguousarray(np.asarray(y, dtype=np.float32))
    conv_w = np.ascontiguousarray(np.asarray(conv_w, dtype=np.float32))
    assert x.shape == (_B, _C, _H, _W), x.shape
    assert y.shape == x.shape
    assert conv_w.shape == (1, 2, 3, 3)

    from concourse.bass_utils import run_bass_kernel_spmd

    nc = _get_nc()
    in_maps = [
        {
            "x": x[i * _BPC : (i + 1) * _BPC],
            "y": y[i * _BPC : (i + 1) * _BPC],
            "conv_w": conv_w,
        }
        for i in range(_NCORES)
    ]
    res = run_bass_kernel_spmd(nc, in_maps, list(range(_NCORES)))
    return np.concatenate([r["out"] for r in res.results], axis=0)


if __name__ == "__main__":
    # quick single-core CoreSim check with bpc=1
    from concourse.bass_interp import CoreSim

    bpc = 1
    nc = _get_nc(bpc)
    rng = np.random.default_rng(0)
    x = rng.standard_normal((bpc, _C, _H, _W), dtype=np.float32)
    y = rng.standard_normal((bpc, _C, _H, _W), dtype=np.float32)
    w = rng.uniform(-0.2, 0.2, (1, 2, 3, 3)).astype(np.float32)

    sim = CoreSim(nc)
    sim.tensor("x")[:] = x
    sim.tensor("y")[:] = y
    sim.tensor("conv_w")[:] = w
    sim.simulate()
    got = sim.tensor("out").copy()

    # numpy reference
    avg = y.mean(axis=1, keepdims=True)
    mx = y.max(axis=1, keepdims=True)
    s = np.concatenate([avg, mx], axis=1)  # [b, 2, H, W]
    sp = np.pad(s, ((0, 0), (0, 0), (1, 1), (1, 1)))
    gate = np.zeros((bpc, _H, _W), dtype=np.float32)
    for ky in range(3):
        for kx in range(3):
            for c in range(2):
                gate += w[0, c, ky, kx] * sp[:, c, ky : ky + _H, kx : kx + _W]
    gate = 1.0 / (1.0 + np.exp(-gate))
    want = x * gate[:, None, :, :]

    err = np.abs(got - want).max()
    rel = err / np.abs(want).max()
    print("max abs err:", err, "rel:", rel)


# revision 19
# speedup vs baseline: 1.0849x; 1.0505x over previous
"""CBAM-style spatial attention kernel for Trainium2 (Bass/Tile), SPMD over 8 cores.

Computation (per batch b):
    avg = mean(y[b], axis=C)            # [H, W]
    mx  = max(y[b], axis=C)             # [H, W]
    gate = sigmoid(conv2d([avg; mx], conv_w, pad=1))   # [H, W]
    out[b] = x[b] * gate                # broadcast over C

Sharding: data-parallel on batch, 4 batches per core, conv_w replicated.

Per-core kernel design:
  - y streamed as [128ch, 2048pix] tiles (2 channel chunks). Each 128-pixel
    subtile is pushed through a PE matmul with moving operand [I | ones/256]
    (129 cols): cols 0..127 give the transpose ([pix, ch] in PSUM), col 128
    gives the per-pixel partial channel mean for free.
  - DVE does one XY max-reduce over the two transposed chunks and one tiny
    X-reduce over the two partial means; planes land transposed (plane_T[w,h]).
  - 3x3 conv = 6 accumulating PE matmuls with banded stationary matrices
    built at runtime from conv_w (dx via band diagonals, dy via shifted
    column windows of the zero-padded planes). Sigmoid on ACT.
  - Gate is PE-transposed back to [h, w], gathered to sig8[8, 2048] (one
    f-tile per partition), then DMA partition-broadcast (step-0) into
    [128, 2048] tiles for the elementwise multiply (DVE for channel chunk A,
    GpSimd for chunk B).
  - DMA queue split to avoid head-of-line blocking: y loads on SP HWDGE,
    x loads on ACT HWDGE, stores + gate broadcasts on Pool SWDGE.
"""

import numpy as np

_B = 32
_NCORES = 8
_BPC = _B // _NCORES  # 4
_C = 256
_H = 128
_W = 128
_HW = _H * _W
_FT = 2048  # pixels per streaming tile
_NFT = _HW // _FT  # 8
_SUBS = _FT // 128  # 16
_GRP = 4  # subtiles per wide PSUM reduce group

_NC_CACHE = {}


def _build(bpc):
    from contextlib import ExitStack

    import concourse.bass as bass
    import concourse.tile as tile
    from concourse import bacc, mybir

    f32 = mybir.dt.float32
    Alu = mybir.AluOpType
    Act = mybir.ActivationFunctionType

    nc = bacc.Bacc("TRN2", target_bir_lowering=False)
    xh = nc.dram_tensor("x", (bpc, _C, _H, _W), f32, kind="ExternalInput")
    yh = nc.dram_tensor("y", (bpc, _C, _H, _W), f32, kind="ExternalInput")
    wh = nc.dram_tensor("conv_w", (1, 2, 3, 3), f32, kind="ExternalInput")
    oh = nc.dram_tensor("out", (bpc, _C, _H, _W), f32, kind="ExternalOutput")
    sig_dram = nc.dram_tensor("sig_scratch", (bpc, _HW), f32)
    sig_ap = sig_dram[:]

    xr = xh[:].rearrange("b c h w -> b c (h w)")
    yr = yh[:].rearrange("b c h w -> b c (h w)")
    orr = oh[:].rearrange("b c h w -> b c (h w)")

    def pbcast(ap, n=128):
        """Partition-broadcast (step-0) view of a [1, F] AP."""
        return bass.AP(
            tensor=ap.tensor,
            offset=ap.offset,
            ap=[[0, n]] + [list(p) for p in ap.ap[1:]],
        )

    with tile.TileContext(nc) as tc, ExitStack() as ctx:
        consts = ctx.enter_context(tc.tile_pool(name="consts", bufs=1))
        ypool = ctx.enter_context(tc.tile_pool(name="ypool", bufs=7))
        xpool = ctx.enter_context(tc.tile_pool(name="xpool", bufs=7))
        gbpool = ctx.enter_context(tc.tile_pool(name="gbpool", bufs=3))
        planep = ctx.enter_context(tc.tile_pool(name="planep", bufs=2))
        smallp = ctx.enter_context(tc.tile_pool(name="smallp", bufs=2))
        tpsum = ctx.enter_context(
            tc.tile_pool(name="tpsum", bufs=3, space=bass.MemorySpace.PSUM)
        )
        gpsum = ctx.enter_context(
            tc.tile_pool(name="gpsum", bufs=1, space=bass.MemorySpace.PSUM)
        )

        # ---- constants ----
        from concourse.masks import make_identity

        identity_t = consts.tile([128, 128], f32)
        make_identity(nc, identity_t[:])
        identity = identity_t[:]

        # conv_w broadcast to all partitions: wbc[p, i] = w.flat[i]
        wbc = consts.tile([128, 18], f32)
        w_flat = wh[:].rearrange("o i kh kw -> o (i kh kw)")  # [1, 18]
        nc.gpsimd.dma_start(out=wbc[:], in_=pbcast(w_flat))
        # avg plane holds the channel SUM; fold mean's 1/C into the avg weights
        nc.scalar.mul(wbc[:, 0:9], wbc[:, 0:9], 1.0 / _C)

        # diagonal shift masks: masks_t[:, j, :][p, f] = 1.0 iff f - p == j - 1
        masks_t = consts.tile([128, 3, 128], f32)
        for j_i, j in enumerate((-1, 0, 1)):
            m = masks_t[:, j_i, :]
            nc.gpsimd.memset(m, 0.0)
            nc.gpsimd.affine_select(
                out=m,
                in_=m,
                compare_op=Alu.not_equal,
                fill=1.0,
                base=-j,
                channel_multiplier=-1,
                pattern=[[1, 128]],
            )

        # banded stationary matrices: band[:, c*3+ky, :][w', w] = W[c, ky, (w-w')+1]
        band = consts.tile([128, 6, 128], f32)
        scr_band = consts.tile([128, 128], f32)
        for c in range(2):
            for ky in range(3):
                idx = c * 3 + ky
                base_i = c * 9 + ky * 3
                # band[p=w', f=w] = W[c, ky, kx] with kx = (p - f) + 1,
                # i.e. mask j = f - p pairs with kx = 1 - j
                bsl = band[:, idx, :]
                nc.vector.tensor_scalar_mul(
                    bsl, masks_t[:, 0, :], wbc[:, base_i + 2 : base_i + 3]
                )
                for j_i, kx in ((1, 1), (2, 0)):
                    nc.vector.tensor_scalar_mul(
                        scr_band[:],
                        masks_t[:, j_i, :],
                        wbc[:, base_i + kx : base_i + kx + 1],
                    )
                    nc.vector.tensor_add(bsl, bsl, scr_band[:])

        # ---- per-batch pipeline ----
        for b in range(bpc):
            # channel-reduced planes, transposed: plane[w, h+1]; cols 0 and H+1 zero
            avgT = planep.tile([128, _H + 2], f32, tag="avgT")
            maxT = planep.tile([128, _H + 2], f32, tag="maxT")
            for t in (avgT, maxT):
                nc.gpsimd.memset(t[:, 0:1], 0.0)
                nc.gpsimd.memset(t[:, _H + 1 : _H + 2], 0.0)

            for ft in range(_NFT):
                sl = slice(ft * _FT, (ft + 1) * _FT)
                ya = ypool.tile([128, _FT], f32, tag="y")
                yb = ypool.tile([128, _FT], f32, tag="y")
                nc.sync.dma_start(out=ya[:], in_=yr[b, 0:128, sl])
                nc.sync.dma_start(out=yb[:], in_=yr[b, 128:256, sl])
                for g in range(_SUBS // _GRP):
                    hcol = ft * _SUBS + g * _GRP + 1  # plane column = h + 1
                    pt = tpsum.tile([128, _GRP, 256], f32, tag="pt")
                    for s_i in range(_GRP):
                        s = g * _GRP + s_i
                        nc.tensor.transpose(
                            pt[:, s_i, 0:128],
                            ya[:, s * 128 : (s + 1) * 128],
                            identity,
                        )
                        nc.tensor.transpose(
                            pt[:, s_i, 128:256],
                            yb[:, s * 128 : (s + 1) * 128],
                            identity,
                        )
                    nc.vector.tensor_reduce(
                        out=maxT[:, hcol : hcol + _GRP],
                        in_=pt[:],
                        axis=mybir.AxisListType.X,
                        op=Alu.max,
                    )
                    nc.vector.tensor_reduce(
                        out=avgT[:, hcol : hcol + _GRP],
                        in_=pt[:],
                        axis=mybir.AxisListType.X,
                        op=Alu.add,
                    )

            # 3x3 conv over [avg; max] planes -> gate_T[w, h] in PSUM
            gate_ps = gpsum.tile([128, 128], f32, tag="gate")
            planes = (avgT, maxT)
            for c in range(2):
                for ky in range(3):
                    idx = c * 3 + ky
                    nc.tensor.matmul(
                        gate_ps[:],
                        band[:, idx, :],
                        planes[c][:, ky : ky + _H],
                        start=(idx == 0),
                        stop=(idx == 5),
                    )
            sigT = smallp.tile([128, 128], f32, tag="sigT")
            nc.scalar.activation(out=sigT[:], in_=gate_ps[:], func=Act.Sigmoid)
            sig_ps = gpsum.tile([128, 128], f32, tag="sigps")
            nc.tensor.matmul(sig_ps[:], sigT[:], identity, start=True, stop=True)
            sighw = smallp.tile([128, 128], f32, tag="sighw")
            nc.scalar.copy(out=sighw[:], in_=sig_ps[:])
            nc.gpsimd.dma_start(out=sig_ap[b, :], in_=sighw[:])

            # multiply: out[b] = x[b] * gate (gate DMA-broadcast across partitions)
            # gb broadcasts are issued 2 f-tiles ahead so Pool's muls never
            # wait on the broadcast DMA they just enqueued
            gbs = []

            def issue_gb(ft):
                sl = slice(ft * _FT, (ft + 1) * _FT)
                gb = gbpool.tile([128, _FT], f32, tag="gb")
                nc.gpsimd.dma_start(out=gb[:], in_=pbcast(sig_ap[b : b + 1, sl]))
                gbs.append(gb)

            issue_gb(0)
            issue_gb(1)
            for ft in range(_NFT):
                sl = slice(ft * _FT, (ft + 1) * _FT)
                if ft + 2 < _NFT:
                    issue_gb(ft + 2)
                gb = gbs[ft]
                xa = xpool.tile([128, _FT], f32, tag="x")
                xb2 = xpool.tile([128, _FT], f32, tag="x")
                nc.scalar.dma_start(out=xa[:], in_=xr[b, 0:128, sl])
                nc.scalar.dma_start(out=xb2[:], in_=xr[b, 128:256, sl])
                nc.gpsimd.tensor_mul(xa[:], xa[:], gb[:])
                nc.gpsimd.tensor_mul(xb2[:], xb2[:], gb[:])
                nc.scalar.dma_start(out=orr[b, 0:128, sl], in_=xa[:])
                nc.scalar.dma_start(out=orr[b, 128:256, sl], in_=xb2[:])

    nc.compile()
    return nc


def _get_nc(bpc=_BPC):
    if bpc not in _NC_CACHE:
        _NC_CACHE[bpc] = _build(bpc)
    return _NC_CACHE[bpc]


def kernel(x, y, conv_w):
    x = np.ascontiguousarray(np.asarray(x, dtype=np.float32))
    y = np.ascontiguousarray(np.asarray(y, dtype=np.float32))
    conv_w = np.ascontiguousarray(np.asarray(conv_w, dtype=np.float32))
    assert x.shape == (_B, _C, _H, _W), x.shape
    assert y.shape == x.shape
    assert conv_w.shape == (1, 2, 3, 3)

    from concourse.bass_utils import run_bass_kernel_spmd

    nc = _get_nc()
    in_maps = [
        {
            "x": x[i * _BPC : (i + 1) * _BPC],
            "y": y[i * _BPC : (i + 1) * _BPC],
            "conv_w": conv_w,
        }
        for i in range(_NCORES)
    ]
    res = run_bass_kernel_spmd(nc, in_maps, list(range(_NCORES)))
    return np.concatenate([r["out"] for r in res.results], axis=0)


if __name__ == "__main__":
    # quick single-core CoreSim check with bpc=1
    from concourse.bass_interp import CoreSim

    bpc = 1
    nc = _get_nc(bpc)
    rng = np.random.default_rng(0)
    x = rng.standard_normal((bpc, _C, _H, _W), dtype=np.float32)
    y = rng.standard_normal((bpc, _C, _H, _W), dtype=np.float32)
    w = rng.uniform(-0.2, 0.2, (1, 2, 3, 3)).astype(np.float32)

    sim = CoreSim(nc)
    sim.tensor("x")[:] = x
    sim.tensor("y")[:] = y
    sim.tensor("conv_w")[:] = w
    sim.simulate()
    got = sim.tensor("out").copy()

    # numpy reference
    avg = y.mean(axis=1, keepdims=True)
    mx = y.max(axis=1, keepdims=True)
    s = np.concatenate([avg, mx], axis=1)  # [b, 2, H, W]
    sp = np.pad(s, ((0, 0), (0, 0), (1, 1), (1, 1)))
    gate = np.zeros((bpc, _H, _W), dtype=np.float32)
    for ky in range(3):
        for kx in range(3):
            for c in range(2):
                gate += w[0, c, ky, kx] * sp[:, c, ky : ky + _H, kx : kx + _W]
    gate = 1.0 / (1.0 + np.exp(-gate))
    want = x * gate[:, None, :, :]

    err = np.abs(got - want).max()
    rel = err / np.abs(want).max()
    print("max abs err:", err, "rel:", rel)
